# revision 20
# baseline (speedup 1.0000x reference)
"""GPTQ int4 linear kernel for Trainium2, 8-way sharded over out_features.

Computes y = x @ W_dq^T + bias where W_dq is group-dequantized from int4
nibbles packed two-per-int32 (only low byte used), with group-wise scales
and zero points:
    even k -> low nibble - 8,  odd k -> high nibble - 8
    W_dq = (W_q * scale[g] + zp[g]),  y = x_perm @ W_dq^T + bias

Device algorithm (per core, shard of 1376 out-rows padded to 1408):
  - qweight viewed as int16 (low half = packed byte v, high half = 0) is
    DMA-xbar-transposed per group into [128 k16, 1408 o] tiles.
  - L = (v & 15) | 0x4300 and H = (v >> 4) | 0x4300, one pure-bitwise
    int16 tensor_scalar op each (walrus forbids bitwise+arith mixing and
    bitwise casts). 0x4300|n is the bf16 bit pattern of 128+n, so a
    bitcast to bf16 yields stationaries with values 128+nibble; the 128
    offset and the -8 nibble centering are removed by a correction matmul
    with rows -136*scale against group-sums of the bf16-ROUNDED x (exact
    cancellation of the big terms). Odd partitions decode to 128.0, so
    the moving operands zero those rows. Matmuls vs x-even / x-odd
    moving operands accumulate per-(group, o-tile) PSUM slots
    [128 o, 32 i].
  - Per-partition scales multiply slots on eviction; a reduce over the
    group axis + a small zp/bias matmul finish y^T.
Host only reshapes/pads inputs (no weight-volume compute) and assembles
the output.
"""

import sys

for _p in ("/opt/trn_rl_repo",):
    if _p not in sys.path:
        sys.path.insert(0, _p)

import numpy as np
import ml_dtypes

import concourse.bacc as bacc
import concourse.bass as bass
import concourse.mybir as mybir
from concourse import tile
from concourse.bass_utils import run_bass_kernel_spmd

# Problem constants (hardcoded per contract)
OUT_F = 11008
IN_F = 4096
GROUP = 128
N_GROUPS = IN_F // GROUP  # 32
BATCH = 32
N_CORES = 8
SHARD = OUT_F // N_CORES      # 1376
SHARD_P = 1408                # padded to a multiple of 128
N_SLOTS = N_GROUPS + 1        # 32 groups + 1 zp/bias slot
CHUNK = 3                     # PSUM slots per chunk (3 banks of 512 f32)
N_CHUNKS = N_SLOTS // CHUNK   # 11

F32 = mybir.dt.float32
BF16 = mybir.dt.bfloat16
I16 = mybir.dt.int16


def build_nc(out_p=SHARD_P, unpack_split=("v", "v"), repeat=1):
    """Build the single-core program (identical across cores, data differs).

    unpack_split: engines for (L-mask, H-shift) per group index parity;
    each element in {"v" (vector), "s" (scalar/ACT), "g" (gpsimd)}.
    repeat: unroll the whole computation N times (for delta timing).
    """
    T = out_p // 128
    nc = bacc.Bacc("TRN2", target_bir_lowering=False, debug=False)

    qw16 = nc.dram_tensor("qw16", [out_p, IN_F], I16, kind="ExternalInput")
    xb_d = nc.dram_tensor("xb", [128, N_GROUPS * BATCH], BF16, kind="ExternalInput")
    xh_d = nc.dram_tensor("xh", [128, N_GROUPS * BATCH], BF16, kind="ExternalInput")
    KZ = 2 * N_GROUPS + 1
    xs_d = nc.dram_tensor("xs", [KZ, BATCH], F32, kind="ExternalInput")
    z_d = nc.dram_tensor("z", [KZ, out_p], F32, kind="ExternalInput")
    sc_d = nc.dram_tensor("scp", [128, T * N_SLOTS], F32, kind="ExternalInput")
    yt_d = nc.dram_tensor("yT", [128, T * BATCH], F32, kind="ExternalOutput")

    def unpack_engine(code):
        return {"v": nc.vector, "s": nc.scalar, "g": nc.gpsimd}[code]

    with tile.TileContext(nc) as tc:
        with (
            tc.tile_pool(name="xc", bufs=1) as xc,
            tc.tile_pool(name="wp", bufs=3) as wp,
            tc.tile_pool(name="acc", bufs=1) as accp,
            tc.tile_pool(name="ps", bufs=2, space="PSUM") as psp,
        ):
            xb = xc.tile([128, N_GROUPS, BATCH], BF16, tag="xb")
            xh = xc.tile([128, N_GROUPS, BATCH], BF16, tag="xh")
            xs = xc.tile([KZ, BATCH], F32, tag="xs")
            z = xc.tile([KZ, out_p], F32, tag="z")
            scp = xc.tile([128, T, N_SLOTS], F32, tag="scp")
            nc.sync.dma_start(xb[:], xb_d[:].rearrange("p (g i) -> p g i", g=N_GROUPS))
            nc.sync.dma_start(xh[:], xh_d[:].rearrange("p (g i) -> p g i", g=N_GROUPS))
            nc.sync.dma_start(xs[:], xs_d[:])
            nc.sync.dma_start(z[:], z_d[:])
            nc.sync.dma_start(scp[:], sc_d[:].rearrange("p (t s) -> p t s", t=T))

            ssc = accp.tile([128, T, BATCH, N_SLOTS], F32, tag="ssc")
            y = accp.tile([128, T, BATCH], F32, tag="y")

            for _rep in range(repeat):
              for c in range(N_CHUNKS):
                ps = psp.tile([128, CHUNK, 512], F32, tag="ps")
                for s in range(CHUNK):
                    gslot = c * CHUNK + s
                    pslot = ps[:, s].rearrange("p (t i) -> p t i", i=BATCH)
                    if gslot < N_GROUPS:
                        g = gslot
                        t16 = wp.tile([128, out_p], I16, tag="t16")
                        nc.sync.dma_start_transpose(
                            out=t16[:], in_=qw16[:, GROUP * g : GROUP * (g + 1)]
                        )
                        bg = wp.tile([128, out_p], I16, tag="bg")
                        hg = wp.tile([128, out_p], I16, tag="hg")
                        eng_b = unpack_engine(unpack_split[0] if g % 2 == 0 else unpack_split[1])
                        eng_h = unpack_engine(unpack_split[1] if g % 2 == 0 else unpack_split[0])
                        # 0x4300 | n is bf16(128 + n) for 0 <= n < 128
                        eng_b.tensor_scalar(
                            bg[:], t16[:], 15, 0x4300,
                            mybir.AluOpType.bitwise_and, mybir.AluOpType.bitwise_or,
                        )
                        eng_h.tensor_scalar(
                            hg[:], t16[:], 4, 0x4300,
                            mybir.AluOpType.logical_shift_right, mybir.AluOpType.bitwise_or,
                        )
                        for t in range(T):
                            nc.tensor.matmul(
                                pslot[:, t],
                                bg[:, t * 128 : (t + 1) * 128].bitcast(BF16),
                                xb[:, g],
                                start=True,
                                stop=False,
                            )
                            nc.tensor.matmul(
                                pslot[:, t],
                                hg[:, t * 128 : (t + 1) * 128].bitcast(BF16),
                                xh[:, g],
                                start=False,
                                stop=True,
                            )
                    else:
                        for t in range(T):
                            nc.tensor.matmul(
                                pslot[:, t],
                                z[:, t * 128 : (t + 1) * 128],
                                xs[:],
                                start=True,
                                stop=True,
                            )
                # Evict chunk: ssc[:, t, i, c*3+s] = ps[p, s, t*32+i] * scales[p, t, slot]
                in0 = (
                    ps[:]
                    .rearrange("p s (t i) -> p s t i", i=BATCH)[:, :, :T]
                    .transpose([0, 2, 3, 1])
                )
                in1 = (
                    scp[:, :, c * CHUNK : (c + 1) * CHUNK]
                    .unsqueeze(2)
                    .broadcast_to([128, T, BATCH, CHUNK])
                )
                nc.vector.tensor_tensor(
                    ssc[:, :, :, c * CHUNK : (c + 1) * CHUNK],
                    in0,
                    in1,
                    mybir.AluOpType.mult,
                )
              nc.vector.tensor_reduce(
                  y[:], ssc[:], axis=mybir.AxisListType.X, op=mybir.AluOpType.add
              )
              nc.sync.dma_start(yt_d[:].rearrange("p (t i) -> p t i", t=T), y[:])

    nc.compile()
    return nc


def prep_inputs(x, qweight_packed, scales, zero_points, bias, perm, out_p=SHARD_P, n_cores=N_CORES):
    """Host-side sharding/reshaping. Only small-tensor compute + views/pads."""
    x = np.asarray(x, np.float32)
    qweight_packed = np.ascontiguousarray(np.asarray(qweight_packed, np.int32))
    scales = np.asarray(scales, np.float32)
    zero_points = np.asarray(zero_points, np.float32)
    bias = np.asarray(bias, np.float32)
    perm = np.asarray(perm, np.int64)
    shard = qweight_packed.shape[0] // n_cores
    T = out_p // 128

    x_perm = x[:, perm]                                  # [B, IN_F]
    xpermT = np.ascontiguousarray(x_perm.T)              # [IN_F, B]
    x3 = xpermT.reshape(N_GROUPS, GROUP, BATCH)          # [g, p, i]
    # moving operands: even partitions carry x_even / x_odd; odd partitions
    # MUST be zero (stationary odd rows hold -8 junk from the int16 view)
    xb3 = np.zeros_like(x3)
    xh3 = np.zeros_like(x3)
    xb3[:, 0::2, :] = x3[:, 0::2, :]   # x at even k
    xh3[:, 0::2, :] = x3[:, 1::2, :]   # x at odd k
    xb_sb = np.ascontiguousarray(xb3.transpose(1, 0, 2)).reshape(128, -1)
    xh_sb = np.ascontiguousarray(xh3.transpose(1, 0, 2)).reshape(128, -1)
    xb_sb = xb_sb.astype(ml_dtypes.bfloat16)
    xh_sb = xh_sb.astype(ml_dtypes.bfloat16)

    # correction rows: zp (vs f32 group sums of x) and -136*scale (vs group
    # sums of the bf16-ROUNDED x actually fed to the PE; 136 = 128 OR-offset
    # + 8 nibble centering) — exact cancellation of the big 128+n terms.
    xsum = x_perm.reshape(BATCH, N_GROUPS, GROUP).sum(-1)          # [i, g] f32
    xsum_r = (
        xb3.astype(ml_dtypes.bfloat16).astype(np.float32).sum(axis=1)
        + xh3.astype(ml_dtypes.bfloat16).astype(np.float32).sum(axis=1)
    )                                                              # [g, i]
    xs = np.concatenate(
        [xsum.T, xsum_r, np.ones((1, BATCH), np.float32)], 0
    ).astype(np.float32)                                           # [65, B]

    qw16_full = qweight_packed.view(np.int16)            # [OUT_F, IN_F], odd cols 0

    in_maps = []
    for c in range(n_cores):
        sl = slice(c * shard, (c + 1) * shard)
        qw16 = np.zeros((out_p, IN_F), np.int16)
        qw16[:shard] = qw16_full[sl]
        z = np.zeros((2 * N_GROUPS + 1, out_p), np.float32)
        z[:N_GROUPS, :shard] = zero_points[sl].T
        z[N_GROUPS : 2 * N_GROUPS, :shard] = (-136.0 * scales[sl]).T
        z[2 * N_GROUPS, :shard] = bias[sl]
        s_pad = np.zeros((out_p, N_GROUPS), np.float32)
        s_pad[:shard] = scales[sl]
        scp = np.concatenate(
            [
                s_pad.reshape(T, 128, N_GROUPS).transpose(1, 0, 2),
                np.ones((128, T, 1), np.float32),
            ],
            axis=2,
        ).reshape(128, -1)
        in_maps.append(
            {
                "qw16": qw16,
                "xb": xb_sb,
                "xh": xh_sb,
                "xs": xs,
                "z": np.ascontiguousarray(z),
                "scp": np.ascontiguousarray(scp),
            }
        )
    return in_maps


def assemble_output(results, out_p=SHARD_P, n_cores=N_CORES, shard=SHARD):
    T = out_p // 128
    cols = []
    for c in range(n_cores):
        yt = np.asarray(results[c]["yT"], np.float32)     # [128, T*B]
        yc = yt.reshape(128, T, BATCH).transpose(2, 1, 0).reshape(BATCH, out_p)
        cols.append(yc[:, :shard])
    return np.concatenate(cols, axis=1)


class _Runner:
    """Builds the program once and keeps one jitted sharded executable so
    repeated calls (and timing loops) reuse the same axon mesh executable."""

    def __init__(self, **build_kwargs):
        import jax
        from jax.sharding import Mesh, PartitionSpec, NamedSharding
        from jax.experimental.shard_map import shard_map
        from concourse import bass2jax

        self.jax = jax
        self.nc = build_nc(**build_kwargs)
        bass2jax.install_neuronx_cc_hook()
        nc = self.nc
        partition_name = (
            nc.partition_id_tensor.name if nc.partition_id_tensor else None
        )
        in_names, out_names, out_avals, zero_outs = [], [], [], []
        for alloc in nc.m.functions[0].allocations:
            if not isinstance(alloc, mybir.MemoryLocationSet):
                continue
            name = alloc.memorylocations[0].name
            if alloc.kind == "ExternalInput":
                if name != partition_name:
                    in_names.append(name)
            elif alloc.kind == "ExternalOutput":
                out_names.append(name)
                shape = tuple(alloc.tensor_shape)
                dtype = mybir.dt.np(alloc.dtype)
                out_avals.append(jax.core.ShapedArray(shape, dtype))
                zero_outs.append(np.zeros(shape, dtype))
        self.in_names, self.out_names = in_names, out_names
        self.out_avals, self.zero_outs = out_avals, zero_outs
        n_params, n_outs = len(in_names), len(out_avals)
        all_names = tuple(in_names + out_names)
        if partition_name is not None:
            all_names = all_names + (partition_name,)

        def _body(*args):
            operands = list(args)
            if partition_name is not None:
                operands.append(bass2jax.partition_id_tensor())
            outs = bass2jax._bass_exec_p.bind(
                *operands,
                out_avals=tuple(out_avals),
                in_names=all_names,
                out_names=tuple(out_names),
                lowering_input_output_aliases=(),
                sim_require_finite=True,
                sim_require_nnan=True,
                nc=nc,
            )
            return tuple(outs)

        devices = jax.devices()[:N_CORES]
        self.mesh = Mesh(np.asarray(devices), ("core",))
        in_specs = (PartitionSpec("core"),) * (n_params + n_outs)
        out_specs = (PartitionSpec("core"),) * n_outs
        self.sharded = jax.jit(
            shard_map(
                _body, mesh=self.mesh, in_specs=in_specs, out_specs=out_specs,
                check_rep=False,
            ),
            donate_argnums=tuple(range(n_params, n_params + n_outs)),
            keep_unused=True,
        )
        self.sharding = NamedSharding(self.mesh, PartitionSpec("core"))

    def put_inputs(self, in_maps):
        jax = self.jax
        arrs = [
            jax.device_put(
                np.concatenate(
                    [np.asarray(in_maps[c][n]) for c in range(N_CORES)], axis=0
                ),
                self.sharding,
            )
            for n in self.in_names
        ]
        for a in arrs:
            a.block_until_ready()
        return arrs

    def execute(self, dev_inputs):
        jax = self.jax
        zs = [
            jax.device_put(
                np.zeros((N_CORES * z.shape[0], *z.shape[1:]), z.dtype), self.sharding
            )
            for z in self.zero_outs
        ]
        for z in zs:
            z.block_until_ready()
        outs = self.sharded(*dev_inputs, *zs)
        jax.block_until_ready(outs)
        return outs

    def run(self, in_maps):
        outs = self.execute(self.put_inputs(in_maps))
        res = []
        for c in range(N_CORES):
            d = {}
            for i, name in enumerate(self.out_names):
                d[name] = np.asarray(outs[i]).reshape(
                    N_CORES, *self.out_avals[i].shape
                )[c]
            res.append(d)
        return res


_RUNNER_CACHE = {}


def get_runner(**build_kwargs):
    key = tuple(sorted(build_kwargs.items()))
    if key not in _RUNNER_CACHE:
        _RUNNER_CACHE[key] = _Runner(**build_kwargs)
    return _RUNNER_CACHE[key]


def kernel(x, qweight_packed, scales, zero_points, bias, perm):
    runner = get_runner()
    in_maps = prep_inputs(x, qweight_packed, scales, zero_points, bias, perm)
    return assemble_output(runner.run(in_maps))
